# revision 2
# baseline (speedup 1.0000x reference)
"""Multi-head self-attention (B=2, S=4096, D=512, H=8, Dh=64) on 8 TRN2 cores.

Sharding: core i handles batch b = i//4 and head-pair hp = i%4 (heads 2*hp,
2*hp+1).  Host sums the 4 partial out-projections per batch.

v6 design (evolved from trace analysis of v2-v5):
- All matmul operands bf16 (PSUM accumulates fp32).  LDWEIGHTS is serialized
  with each matmul on this toolchain (--enable-ldw-opt=false, no FWL), so
  fewer/cheaper weight loads matter.
- QB=512 with BOTH heads processed per (qb, k) unit:
  * scores: two row-tiled matmuls (K=64 each) run CONCURRENTLY in the PE
    array -- h0 in rows 0-63 (tile_position (0,0)), h1 in rows 64-127
    ((64,0)), writing halves of one shared s2 [128,1024] PSUM tile.
  * ONE [128,1024] exp covers both heads (ACT is the bottleneck engine;
    1 elem/lane/cycle @ 1.2 GHz regardless of dtype).
  * ctx: per-head [128,65] x [128,512] accumulating matmuls (vones carries a
    ones column; row 64 of ctx PSUM = softmax denominator).
- One FLAT software pipeline over all 256 (qb, k) units -- scores run 2 units
  ahead, crossing qb boundaries with no pipeline drain (PE idle gaps >3.4us
  trip the HAM MID window and throttle the PE clock 2.4->1.2 GHz).
- Out-projection of qb-1 and Q-projection of qb+1 are spread inside qb's
  unit loop at different k so PSUM slot reuse never stalls the PE on DVE.
- Per-unit PE work (~1.05us) ~= ACT exp (~1.11us): ACT-bound steady state
  with near-zero PE idle.
- PSUM: s2 [128,1024] x2 (4 banks) + ctx0 [128,512] x2 + ctx1 [128,512] x2
  (4 banks) = 8 banks.

TRN2 quirk: walrus encodes exactly ONE sync wait per TPB compute
instruction; _legalize_matmul_waits moves extra waits onto injected
single-wait same-engine no-ops.
"""

import sys
from contextlib import ExitStack

for _p in ("/opt/trn_rl_repo",):
    if _p not in sys.path:
        sys.path.insert(0, _p)

import numpy as np

import concourse.bass as bass
import concourse.tile as tile
from concourse import mybir
from concourse.bass_utils import run_bass_kernel_spmd

F32 = mybir.dt.float32
BF16 = mybir.dt.bfloat16
D = 512
DH = 64
P = 128
B = 2
S_FULL = 4096
N_CORES = 8
NC_T = D // P

LAST_RESULTS = None


def _emit(nc: bass.Bass, tc: "tile.TileContext", ctx: ExitStack, S: int):
    NK = S // P                  # 128-key tiles
    QB = 512 if S >= 512 else S
    NQB = S // QB                # q-blocks of 512
    U = NQB * NK                 # total pipeline units
    BS = min(1024, S)            # proj block size
    NB = S // BS
    KT_B = BS // P               # k-tiles per proj block
    inv_scale = 1.0 / np.sqrt(DH)

    def mm(out, lhsT, rhs, start=True, stop=True):
        return nc.tensor.matmul(out, lhsT, rhs, start=start, stop=stop)

    xt = nc.declare_dram_parameter("xt", [D, S], BF16, isOutput=False)
    wq = nc.declare_dram_parameter("wq", [D, P], BF16, isOutput=False)
    wk = nc.declare_dram_parameter("wk", [D, P], BF16, isOutput=False)
    wv = nc.declare_dram_parameter("wv", [D, P], BF16, isOutput=False)
    wo = nc.declare_dram_parameter("wo", [P, D], BF16, isOutput=False)
    yt = nc.declare_dram_parameter("yt", [D, S], F32, isOutput=True)

    const = ctx.enter_context(tc.tile_pool(name="const", bufs=1))

    # ---- input DMAs (block-major so proj block b can start early) ----
    w_sb = {}
    for name, ap in (("wq", wq), ("wk", wk), ("wv", wv)):
        tiles = []
        for c in range(NC_T):
            t = const.tile([P, P], BF16, tag=f"{name}{c}", name=f"{name}{c}")
            nc.sync.dma_start(out=t[:], in_=ap[c * P:(c + 1) * P, :])
            tiles.append(t)
        w_sb[name] = tiles
    wo_sb = const.tile([P, D], BF16, tag="wo")
    nc.sync.dma_start(out=wo_sb[:], in_=wo[:, :])
    xt_sb = [const.tile([P, S], BF16, tag=f"xt{c}", name=f"xt{c}")
             for c in range(NC_T)]
    for b in range(NB):
        bsl = slice(b * BS, (b + 1) * BS)
        for c in range(NC_T):
            nc.sync.dma_start(out=xt_sb[c][:, bsl], in_=xt[c * P:(c + 1) * P, bsl])

    # ---- persistent SBUF ----
    qt_sb = const.tile([P, S], BF16, tag="qt")     # [2x64 dh, S]
    kt_sb = const.tile([P, S], BF16, tag="kt")
    vones = [const.tile([P, NK * (DH + 1)], BF16, tag=f"vones{h}", name=f"vones{h}")
             for h in range(2)]
    konst = const.tile([P, NK, 1], F32, tag="konst")
    nc.vector.memset(konst[:], 1.0)
    for h in range(2):
        vv = vones[h].rearrange("p (k c) -> p k c", c=DH + 1)
        nc.vector.tensor_copy(vv[:, :, DH:DH + 1], konst[:])
    ctx_sb = const.tile([P, S], BF16, tag="ctx")   # normalized context^T

    # PSUM pools: s2 2x[128,1024] (4 banks) + ctx0/ctx1 2x[128,512] each (4)
    ps = ctx.enter_context(tc.tile_pool(name="ps", bufs=2, space="PSUM"))
    es = ctx.enter_context(tc.tile_pool(name="es", bufs=3))
    bcp = ctx.enter_context(tc.tile_pool(name="bcp", bufs=2))
    rtp = ctx.enter_context(tc.tile_pool(name="rtp", bufs=2))
    rdp = ctx.enter_context(tc.tile_pool(name="rdp", bufs=2, space="DRAM"))
    osb = ctx.enter_context(tc.tile_pool(name="osb", bufs=2))

    # ---- phase A: V + K projections per block, Q for qb0(+qb1) ----
    def emit_vproj_block(b):
        pv = ps.tile([P, 2 * QB], F32, tag="s2", name=f"pv{b}")
        for j in range(KT_B):
            k = b * KT_B + j
            ksl = slice(k * P, (k + 1) * P)
            for c in range(NC_T):
                mm(pv[:, j * P:(j + 1) * P], xt_sb[c][:, ksl], w_sb["wv"][c][:],
                   start=(c == 0), stop=(c == NC_T - 1))
        for h in range(2):
            src = pv[:, 0:KT_B * P].rearrange("p (j c) -> p j c", c=P)[:, :, h * DH:(h + 1) * DH]
            dst = vones[h].rearrange("p (k c) -> p k c", c=DH + 1)[
                :, b * KT_B:(b + 1) * KT_B, 0:DH]
            nc.vector.tensor_copy(dst, src)

    def emit_kproj_block(b):
        bsl = slice(b * BS, (b + 1) * BS)
        pt = ps.tile([P, 2 * QB], F32, tag="s2", name=f"pk{b}")
        nh = max(1, BS // 512)
        hw_ = BS // nh
        for j in range(nh):
            jsl = slice(b * BS + j * hw_, b * BS + (j + 1) * hw_)
            for c in range(NC_T):
                mm(pt[:, j * hw_:(j + 1) * hw_], w_sb["wk"][c][:],
                   xt_sb[c][:, jsl], start=(c == 0), stop=(c == NC_T - 1))
        nc.vector.tensor_copy(kt_sb[:, bsl], pt[:, 0:BS])

    def emit_qproj_qb(qb):
        """Q projection for one 512-wide q-block into half an s2 tile."""
        qsl = slice(qb * QB, (qb + 1) * QB)
        pt = ps.tile([P, 2 * QB], F32, tag="s2", name=f"pq{qb}")
        for c in range(NC_T):
            mm(pt[:, 0:QB], w_sb["wq"][c][:], xt_sb[c][:, qsl],
               start=(c == 0), stop=(c == NC_T - 1))
        nc.vector.tensor_copy(qt_sb[:, qsl], pt[:, 0:QB])

    def emit_outproj_pair(qb, epair):
        """Out-projection for q-block qb, e-blocks (2*epair, 2*epair+1)."""
        qsl = slice(qb * QB, (qb + 1) * QB)
        o_ps = ps.tile([P, 2 * QB], F32, tag="s2", name=f"o{qb}_{epair}")
        for i in range(2):
            e = 2 * epair + i
            mm(o_ps[:, i * QB:(i + 1) * QB],
               wo_sb[:, e * P:(e + 1) * P], ctx_sb[:, qsl])
        o_sb = osb.tile([P, 2 * QB], F32, tag="osb", name="o_sb")
        nc.vector.tensor_copy(o_sb[:], o_ps[:, 0:2 * QB])
        for i in range(2):
            e = 2 * epair + i
            nc.sync.dma_start(out=yt[e * P:(e + 1) * P, qsl],
                              in_=o_sb[:, i * QB:(i + 1) * QB])

    for b in range(NB):
        emit_vproj_block(b)
        emit_kproj_block(b)
    emit_qproj_qb(0)
    if NQB > 1:
        emit_qproj_qb(1)

    # ---- attention: flat pipeline over units u = qb*NK + k ----
    def emit_scores_pair(u, s2):
        qb, k = divmod(u, NK)
        qsl = slice(qb * QB, (qb + 1) * QB)
        ksl = slice(k * P, (k + 1) * P)
        for h in range(2):
            hsl = slice(h * DH, (h + 1) * DH)
            mm(s2[:, h * QB:(h + 1) * QB], kt_sb[hsl, ksl], qt_sb[hsl, qsl])

    s_tiles = {}
    ctx_tiles = [None, None]     # live ctx PSUM tile per head
    for u in range(min(2, U)):
        s_tiles[u] = ps.tile([P, 2 * QB], F32, tag="s2", name=f"s{u}")
        emit_scores_pair(u, s_tiles[u])

    for u in range(U):
        qb, k = divmod(u, NK)
        if k == 0:
            for h in range(2):
                ctx_tiles[h] = ps.tile([P, QB], F32, tag=f"ctx{h}",
                                       name=f"ctx{qb}_{h}")
        # ACT: one exp for both heads
        e2 = es.tile([P, 2 * QB], BF16, tag="e", name=f"e{u}")
        nc.scalar.activation(e2[:], s_tiles.pop(u)[:],
                             mybir.ActivationFunctionType.Exp,
                             scale=inv_scale)
        # PE: ctx accumulation per head
        for h in range(2):
            vo = vones[h][:, k * (DH + 1):(k + 1) * (DH + 1)]
            mm(ctx_tiles[h][0:DH + 1, :], vo, e2[:, h * QB:(h + 1) * QB],
               start=(k == 0), stop=(k == NK - 1))
        # insertions (spread; never adjacent to each other)
        if k == 6 and qb + 2 < NQB:
            emit_qproj_qb(qb + 2)
        elif k == 14 and qb > 0:
            emit_outproj_pair(qb - 1, 0)
        elif k == 22 and qb > 0:
            emit_outproj_pair(qb - 1, 1)
        # PE: scores for unit u+2
        if u + 2 < U:
            s_tiles[u + 2] = ps.tile([P, 2 * QB], F32, tag="s2",
                                     name=f"s{u + 2}")
            emit_scores_pair(u + 2, s_tiles[u + 2])
        # drains at the end of each q-block
        if k == NK - 1:
            qsl = slice(qb * QB, (qb + 1) * QB)
            for h in range(2):
                hsl = slice(h * DH, (h + 1) * DH)
                cps = ctx_tiles[h]
                FW = QB // DH            # 8 free elems per lane
                # denominator row -> DRAM -> [64, FW] lanes -> recip (fast,
                # 64-lane) -> DRAM -> partition-broadcast [DH, QB]
                rt0 = rtp.tile([1, QB], F32, tag="rt0", name="rt0")
                nc.vector.tensor_copy(rt0[0:1, :], cps[DH:DH + 1, :])
                rtd = rdp.tile([1, QB], F32, tag="rtd", name="rtd")
                nc.sync.dma_start(out=rtd[:], in_=rt0[0:1, :])
                rtd_lanes = bass.AP(tensor=rtd.tensor, offset=rtd.offset,
                                    ap=[[FW, DH], [1, FW]])
                rtT = rtp.tile([DH, FW], F32, tag="rt", name="rt")
                nc.sync.dma_start(out=rtT[:], in_=rtd_lanes)
                nc.vector.reciprocal(rtT[:], rtT[:])
                rtd2 = rdp.tile([1, QB], F32, tag="rtd2", name="rtd2")
                rtd2_lanes = bass.AP(tensor=rtd2.tensor, offset=rtd2.offset,
                                     ap=[[FW, DH], [1, FW]])
                nc.sync.dma_start(out=rtd2_lanes, in_=rtT[:])
                bc = bcp.tile([DH, QB], F32, tag="bc", name="bc")
                rtd2_bcast = bass.AP(tensor=rtd2.tensor, offset=rtd2.offset,
                                     ap=[[0, DH], [1, QB]])
                nc.sync.dma_start(out=bc[:], in_=rtd2_bcast)
                nc.vector.tensor_mul(ctx_sb[hsl, qsl], cps[0:DH, :], bc[:])

    emit_outproj_pair(NQB - 1, 0)
    emit_outproj_pair(NQB - 1, 1)


def _legalize_matmul_waits(nc: bass.Bass) -> int:
    n_fixed = 0
    for f in nc.m.functions:
        for bb in f.blocks:
            out = []
            changed = False
            for ins in bb.instructions:
                si = ins.sync_info
                if (getattr(ins, "engine", None) is not None
                        and si is not None and len(si.on_wait) > 1):
                    for idx, w in enumerate(si.on_wait[:-1]):
                        nop = mybir.InstNoOp(name=f"{ins.name}-lgw{idx}",
                                             ins=[], outs=[])
                        nop.engine = ins.engine
                        nop.sync_info = mybir.SyncInfo(on_wait=[w], on_update=[])
                        out.append(nop)
                    ins.sync_info = mybir.SyncInfo(on_wait=[si.on_wait[-1]],
                                                   on_update=si.on_update)
                    n_fixed += 1
                    changed = True
                out.append(ins)
            if changed:
                bb.instructions = out
    return n_fixed


def build(S: int = S_FULL, legalize: bool = False) -> bass.Bass:
    nc = bass.Bass()
    with ExitStack() as ctx:
        ctx.enter_context(nc.allow_low_precision(
            reason="bf16 matmul operands; fp32 PSUM accumulation"))
        tc = ctx.enter_context(tile.TileContext(nc))
        _emit(nc, tc, ctx, S)
    if legalize:
        _legalize_matmul_waits(nc)
    return nc


_NC_CACHE = {}


def _get_nc(S: int) -> bass.Bass:
    if S not in _NC_CACHE:
        _NC_CACHE[S] = build(S, legalize=True)
    return _NC_CACHE[S]


def _to_bf16(a):
    import ml_dtypes
    return np.ascontiguousarray(a, dtype=np.float32).astype(ml_dtypes.bfloat16)


def make_in_maps(X, Wq, Wk, Wv, Wo):
    xts = [_to_bf16(X[b].T) for b in range(B)]
    in_maps = []
    for i in range(N_CORES):
        b, hp = divmod(i, 4)
        csl = slice(hp * P, (hp + 1) * P)
        in_maps.append({
            "xt": xts[b],
            "wq": _to_bf16(Wq[:, csl]),
            "wk": _to_bf16(Wk[:, csl]),
            "wv": _to_bf16(Wv[:, csl]),
            "wo": _to_bf16(Wo[csl, :]),
        })
    return in_maps


def kernel(X, Wq, Wk, Wv, Wo, _trace=False):
    global LAST_RESULTS
    X = np.asarray(X, dtype=np.float32)
    S = X.shape[1]
    nc = _get_nc(S)
    in_maps = make_in_maps(X, np.asarray(Wq, np.float32), np.asarray(Wk, np.float32),
                           np.asarray(Wv, np.float32), np.asarray(Wo, np.float32))
    res = run_bass_kernel_spmd(nc, in_maps, list(range(N_CORES)), trace=_trace)
    LAST_RESULTS = res
    Y = np.zeros((B, S, D), dtype=np.float32)
    for i in range(N_CORES):
        Y[i // 4] += res.results[i]["yt"].T
    return Y


# revision 3
# speedup vs baseline: 1.1799x; 1.1799x over previous
"""Multi-head self-attention (B=2, S=4096, D=512, H=8, Dh=64) on 8 TRN2 cores.

Sharding: core i handles batch b = i//4 and head-pair hp = i%4 (heads 2*hp,
2*hp+1).  Host sums the 4 partial out-projections per batch.

v6 design (evolved from trace analysis of v2-v5):
- All matmul operands bf16 (PSUM accumulates fp32).  LDWEIGHTS is serialized
  with each matmul on this toolchain (--enable-ldw-opt=false, no FWL), so
  fewer/cheaper weight loads matter.
- QB=512 with BOTH heads processed per (qb, k) unit:
  * scores: two row-tiled matmuls (K=64 each) run CONCURRENTLY in the PE
    array -- h0 in rows 0-63 (tile_position (0,0)), h1 in rows 64-127
    ((64,0)), writing halves of one shared s2 [128,1024] PSUM tile.
  * ONE [128,1024] exp covers both heads (ACT is the bottleneck engine;
    1 elem/lane/cycle @ 1.2 GHz regardless of dtype).
  * ctx: per-head [128,65] x [128,512] accumulating matmuls (vones carries a
    ones column; row 64 of ctx PSUM = softmax denominator).
- One FLAT software pipeline over all 256 (qb, k) units -- scores run 2 units
  ahead, crossing qb boundaries with no pipeline drain (PE idle gaps >3.4us
  trip the HAM MID window and throttle the PE clock 2.4->1.2 GHz).
- Out-projection of qb-1 and Q-projection of qb+2 are spread inside qb's
  unit loop at different k so PSUM slot reuse never stalls the PE on DVE.
- Per-unit PE work (~1.05us) ~= ACT exp (~1.11us): ACT-bound steady state
  with near-zero PE idle.
- PSUM: s2 [128,1024] x2 (4 banks) + ctx0 [128,512] x2 + ctx1 [128,512] x2
  (4 banks) = 8 banks.
- Softmax denominators: single-lane DVE reciprocal ([1,512] = 3.3us, 8
  cyc/elem) would back up the DVE queue and stall the PE via PSUM-slot
  reuse; instead DMA the row to [64,8] lanes, reciprocal there (194ns),
  then DRAM-bounce a partition-broadcast (SBUF APs can't stride-0).
- Weight DMAs issue before the bulk xt chunk DMAs (each dma_start costs
  ~0.6us serialized per queue; V-proj needs wv + xt block 0 first).

TRN2 quirk: walrus encodes exactly ONE sync wait per TPB compute
instruction; _legalize_matmul_waits moves extra waits onto injected
single-wait same-engine no-ops.
"""

import sys
from contextlib import ExitStack

for _p in ("/opt/trn_rl_repo",):
    if _p not in sys.path:
        sys.path.insert(0, _p)

import numpy as np

import concourse.bass as bass
import concourse.tile as tile
from concourse import mybir
from concourse.bass_utils import run_bass_kernel_spmd

F32 = mybir.dt.float32
BF16 = mybir.dt.bfloat16
D = 512
DH = 64
P = 128
B = 2
S_FULL = 4096
N_CORES = 8
NC_T = D // P

LAST_RESULTS = None


def _emit(nc: bass.Bass, tc: "tile.TileContext", ctx: ExitStack, S: int):
    NK = S // P                  # 128-key tiles
    QB = 512 if S >= 512 else S
    NQB = S // QB                # q-blocks of 512
    U = NQB * NK                 # total pipeline units
    BS = min(1024, S)            # proj block size
    NB = S // BS
    KT_B = BS // P               # k-tiles per proj block
    inv_scale = 1.0 / np.sqrt(DH)

    def mm(out, lhsT, rhs, start=True, stop=True):
        return nc.tensor.matmul(out, lhsT, rhs, start=start, stop=stop)

    xt = nc.declare_dram_parameter("xt", [D, S], BF16, isOutput=False)
    wq = nc.declare_dram_parameter("wq", [D, P], BF16, isOutput=False)
    wk = nc.declare_dram_parameter("wk", [D, P], BF16, isOutput=False)
    wv = nc.declare_dram_parameter("wv", [D, P], BF16, isOutput=False)
    wo = nc.declare_dram_parameter("wo", [P, D], BF16, isOutput=False)
    yt = nc.declare_dram_parameter("yt", [D, S], F32, isOutput=True)

    const = ctx.enter_context(tc.tile_pool(name="const", bufs=1))

    # ---- input DMAs (block-major so proj block b can start early) ----
    w_sb = {}
    for name, ap in (("wq", wq), ("wk", wk), ("wv", wv)):
        tiles = []
        for c in range(NC_T):
            t = const.tile([P, P], BF16, tag=f"{name}{c}", name=f"{name}{c}")
            nc.sync.dma_start(out=t[:], in_=ap[c * P:(c + 1) * P, :])
            tiles.append(t)
        w_sb[name] = tiles
    wo_sb = const.tile([P, D], BF16, tag="wo")
    nc.sync.dma_start(out=wo_sb[:], in_=wo[:, :])
    xt_sb = [const.tile([P, S], BF16, tag=f"xt{c}", name=f"xt{c}")
             for c in range(NC_T)]
    for b in range(NB):
        bsl = slice(b * BS, (b + 1) * BS)
        for c in range(NC_T):
            nc.sync.dma_start(out=xt_sb[c][:, bsl], in_=xt[c * P:(c + 1) * P, bsl])

    # ---- persistent SBUF ----
    qt_sb = const.tile([P, S], BF16, tag="qt")     # [2x64 dh, S]
    kt_sb = const.tile([P, S], BF16, tag="kt")
    vones = [const.tile([P, NK * (DH + 1)], BF16, tag=f"vones{h}", name=f"vones{h}")
             for h in range(2)]
    konst = const.tile([P, NK, 1], F32, tag="konst")
    nc.vector.memset(konst[:], 1.0)
    for h in range(2):
        vv = vones[h].rearrange("p (k c) -> p k c", c=DH + 1)
        nc.vector.tensor_copy(vv[:, :, DH:DH + 1], konst[:])
    ctx_sb = const.tile([P, S], BF16, tag="ctx")   # normalized context^T

    # PSUM pools: s2 2x[128,1024] (4 banks) + ctx0/ctx1 2x[128,512] each (4)
    ps = ctx.enter_context(tc.tile_pool(name="ps", bufs=2, space="PSUM"))
    es = ctx.enter_context(tc.tile_pool(name="es", bufs=3))
    bcp = ctx.enter_context(tc.tile_pool(name="bcp", bufs=2))
    rtp = ctx.enter_context(tc.tile_pool(name="rtp", bufs=2))
    rdp = ctx.enter_context(tc.tile_pool(name="rdp", bufs=2, space="DRAM"))
    osb = ctx.enter_context(tc.tile_pool(name="osb", bufs=2))

    # ---- phase A: V + K projections per block, Q for qb0(+qb1) ----
    def emit_vproj_block(b):
        pv = ps.tile([P, 2 * QB], F32, tag="s2", name=f"pv{b}")
        for j in range(KT_B):
            k = b * KT_B + j
            ksl = slice(k * P, (k + 1) * P)
            for c in range(NC_T):
                mm(pv[:, j * P:(j + 1) * P], xt_sb[c][:, ksl], w_sb["wv"][c][:],
                   start=(c == 0), stop=(c == NC_T - 1))
        for h in range(2):
            src = pv[:, 0:KT_B * P].rearrange("p (j c) -> p j c", c=P)[:, :, h * DH:(h + 1) * DH]
            dst = vones[h].rearrange("p (k c) -> p k c", c=DH + 1)[
                :, b * KT_B:(b + 1) * KT_B, 0:DH]
            nc.vector.tensor_copy(dst, src)

    def emit_kproj_block(b):
        bsl = slice(b * BS, (b + 1) * BS)
        pt = ps.tile([P, 2 * QB], F32, tag="s2", name=f"pk{b}")
        nh = max(1, BS // 512)
        hw_ = BS // nh
        for j in range(nh):
            jsl = slice(b * BS + j * hw_, b * BS + (j + 1) * hw_)
            for c in range(NC_T):
                mm(pt[:, j * hw_:(j + 1) * hw_], w_sb["wk"][c][:],
                   xt_sb[c][:, jsl], start=(c == 0), stop=(c == NC_T - 1))
        nc.vector.tensor_copy(kt_sb[:, bsl], pt[:, 0:BS])

    def emit_qproj_qb(qb):
        """Q projection for one 512-wide q-block into half an s2 tile."""
        qsl = slice(qb * QB, (qb + 1) * QB)
        pt = ps.tile([P, 2 * QB], F32, tag="s2", name=f"pq{qb}")
        for c in range(NC_T):
            mm(pt[:, 0:QB], w_sb["wq"][c][:], xt_sb[c][:, qsl],
               start=(c == 0), stop=(c == NC_T - 1))
        nc.vector.tensor_copy(qt_sb[:, qsl], pt[:, 0:QB])

    def emit_outproj_pair(qb, epair):
        """Out-projection for q-block qb, e-blocks (2*epair, 2*epair+1)."""
        qsl = slice(qb * QB, (qb + 1) * QB)
        o_ps = ps.tile([P, 2 * QB], F32, tag="s2", name=f"o{qb}_{epair}")
        for i in range(2):
            e = 2 * epair + i
            mm(o_ps[:, i * QB:(i + 1) * QB],
               wo_sb[:, e * P:(e + 1) * P], ctx_sb[:, qsl])
        o_sb = osb.tile([P, 2 * QB], F32, tag="osb", name="o_sb")
        nc.vector.tensor_copy(o_sb[:], o_ps[:, 0:2 * QB])
        for i in range(2):
            e = 2 * epair + i
            nc.sync.dma_start(out=yt[e * P:(e + 1) * P, qsl],
                              in_=o_sb[:, i * QB:(i + 1) * QB])

    for b in range(NB):
        emit_vproj_block(b)
        emit_kproj_block(b)
    emit_qproj_qb(0)
    if NQB > 1:
        emit_qproj_qb(1)

    # ---- attention: flat pipeline over units u = qb*NK + k ----
    def emit_scores_pair(u, s2):
        qb, k = divmod(u, NK)
        qsl = slice(qb * QB, (qb + 1) * QB)
        ksl = slice(k * P, (k + 1) * P)
        for h in range(2):
            hsl = slice(h * DH, (h + 1) * DH)
            mm(s2[:, h * QB:(h + 1) * QB], kt_sb[hsl, ksl], qt_sb[hsl, qsl])

    s_tiles = {}
    ctx_tiles = [None, None]     # live ctx PSUM tile per head
    for u in range(min(2, U)):
        s_tiles[u] = ps.tile([P, 2 * QB], F32, tag="s2", name=f"s{u}")
        emit_scores_pair(u, s_tiles[u])

    for u in range(U):
        qb, k = divmod(u, NK)
        if k == 0:
            for h in range(2):
                ctx_tiles[h] = ps.tile([P, QB], F32, tag=f"ctx{h}",
                                       name=f"ctx{qb}_{h}")
        # ACT: one exp for both heads
        e2 = es.tile([P, 2 * QB], BF16, tag="e", name=f"e{u}")
        nc.scalar.activation(e2[:], s_tiles.pop(u)[:],
                             mybir.ActivationFunctionType.Exp,
                             scale=inv_scale)
        # PE: ctx accumulation per head
        for h in range(2):
            vo = vones[h][:, k * (DH + 1):(k + 1) * (DH + 1)]
            mm(ctx_tiles[h][0:DH + 1, :], vo, e2[:, h * QB:(h + 1) * QB],
               start=(k == 0), stop=(k == NK - 1))
        # insertions (spread; never adjacent to each other)
        if k == 6 and qb + 2 < NQB:
            emit_qproj_qb(qb + 2)
        elif k == 14 and qb > 0:
            emit_outproj_pair(qb - 1, 0)
        elif k == 22 and qb > 0:
            emit_outproj_pair(qb - 1, 1)
        # PE: scores for unit u+2
        if u + 2 < U:
            s_tiles[u + 2] = ps.tile([P, 2 * QB], F32, tag="s2",
                                     name=f"s{u + 2}")
            emit_scores_pair(u + 2, s_tiles[u + 2])
        # drains at the end of each q-block
        if k == NK - 1:
            qsl = slice(qb * QB, (qb + 1) * QB)
            for h in range(2):
                hsl = slice(h * DH, (h + 1) * DH)
                cps = ctx_tiles[h]
                FW = QB // DH            # 8 free elems per lane
                # denominator row -> DRAM -> [64, FW] lanes -> recip (fast,
                # 64-lane) -> DRAM -> partition-broadcast [DH, QB]
                rt0 = rtp.tile([1, QB], F32, tag="rt0", name="rt0")
                nc.vector.tensor_copy(rt0[0:1, :], cps[DH:DH + 1, :])
                rtd = rdp.tile([1, QB], F32, tag="rtd", name="rtd")
                nc.sync.dma_start(out=rtd[:], in_=rt0[0:1, :])
                rtd_lanes = bass.AP(tensor=rtd.tensor, offset=rtd.offset,
                                    ap=[[FW, DH], [1, FW]])
                rtT = rtp.tile([DH, FW], F32, tag="rt", name="rt")
                nc.sync.dma_start(out=rtT[:], in_=rtd_lanes)
                nc.vector.reciprocal(rtT[:], rtT[:])
                rtd2 = rdp.tile([1, QB], F32, tag="rtd2", name="rtd2")
                rtd2_lanes = bass.AP(tensor=rtd2.tensor, offset=rtd2.offset,
                                     ap=[[FW, DH], [1, FW]])
                nc.sync.dma_start(out=rtd2_lanes, in_=rtT[:])
                bc = bcp.tile([DH, QB], F32, tag="bc", name="bc")
                rtd2_bcast = bass.AP(tensor=rtd2.tensor, offset=rtd2.offset,
                                     ap=[[0, DH], [1, QB]])
                nc.sync.dma_start(out=bc[:], in_=rtd2_bcast)
                nc.vector.tensor_mul(ctx_sb[hsl, qsl], cps[0:DH, :], bc[:])

    emit_outproj_pair(NQB - 1, 0)
    emit_outproj_pair(NQB - 1, 1)


def _legalize_matmul_waits(nc: bass.Bass) -> int:
    n_fixed = 0
    for f in nc.m.functions:
        for bb in f.blocks:
            out = []
            changed = False
            for ins in bb.instructions:
                si = ins.sync_info
                if (getattr(ins, "engine", None) is not None
                        and si is not None and len(si.on_wait) > 1):
                    for idx, w in enumerate(si.on_wait[:-1]):
                        nop = mybir.InstNoOp(name=f"{ins.name}-lgw{idx}",
                                             ins=[], outs=[])
                        nop.engine = ins.engine
                        nop.sync_info = mybir.SyncInfo(on_wait=[w], on_update=[])
                        out.append(nop)
                    ins.sync_info = mybir.SyncInfo(on_wait=[si.on_wait[-1]],
                                                   on_update=si.on_update)
                    n_fixed += 1
                    changed = True
                out.append(ins)
            if changed:
                bb.instructions = out
    return n_fixed


def build(S: int = S_FULL, legalize: bool = False) -> bass.Bass:
    nc = bass.Bass()
    with ExitStack() as ctx:
        ctx.enter_context(nc.allow_low_precision(
            reason="bf16 matmul operands; fp32 PSUM accumulation"))
        tc = ctx.enter_context(tile.TileContext(nc))
        _emit(nc, tc, ctx, S)
    if legalize:
        _legalize_matmul_waits(nc)
    return nc


_NC_CACHE = {}


def _get_nc(S: int) -> bass.Bass:
    if S not in _NC_CACHE:
        _NC_CACHE[S] = build(S, legalize=True)
    return _NC_CACHE[S]


def _to_bf16(a):
    import ml_dtypes
    return np.ascontiguousarray(a, dtype=np.float32).astype(ml_dtypes.bfloat16)


def make_in_maps(X, Wq, Wk, Wv, Wo):
    xts = [_to_bf16(X[b].T) for b in range(B)]
    in_maps = []
    for i in range(N_CORES):
        b, hp = divmod(i, 4)
        csl = slice(hp * P, (hp + 1) * P)
        in_maps.append({
            "xt": xts[b],
            "wq": _to_bf16(Wq[:, csl]),
            "wk": _to_bf16(Wk[:, csl]),
            "wv": _to_bf16(Wv[:, csl]),
            "wo": _to_bf16(Wo[csl, :]),
        })
    return in_maps


def kernel(X, Wq, Wk, Wv, Wo, _trace=False):
    global LAST_RESULTS
    X = np.asarray(X, dtype=np.float32)
    S = X.shape[1]
    nc = _get_nc(S)
    in_maps = make_in_maps(X, np.asarray(Wq, np.float32), np.asarray(Wk, np.float32),
                           np.asarray(Wv, np.float32), np.asarray(Wo, np.float32))
    res = run_bass_kernel_spmd(nc, in_maps, list(range(N_CORES)), trace=_trace)
    LAST_RESULTS = res
    Y = np.zeros((B, S, D), dtype=np.float32)
    for i in range(N_CORES):
        Y[i // 4] += res.results[i]["yt"].T
    return Y


# revision 4
# speedup vs baseline: 1.1927x; 1.0109x over previous
"""Multi-head self-attention (B=2, S=4096, D=512, H=8, Dh=64) on 8 TRN2 cores.

Sharding: core i handles batch b = i//4 and head-pair hp = i%4 (heads 2*hp,
2*hp+1).  Host sums the 4 partial out-projections per batch.

v6 design (evolved from trace analysis of v2-v5):
- All matmul operands bf16 (PSUM accumulates fp32).  LDWEIGHTS is serialized
  with each matmul on this toolchain (--enable-ldw-opt=false, no FWL), so
  fewer/cheaper weight loads matter.
- QB=512 with BOTH heads processed per (qb, k) unit:
  * scores: two row-tiled matmuls (K=64 each) run CONCURRENTLY in the PE
    array -- h0 in rows 0-63 (tile_position (0,0)), h1 in rows 64-127
    ((64,0)), writing halves of one shared s2 [128,1024] PSUM tile.
  * ONE [128,1024] exp covers both heads (ACT is the bottleneck engine;
    1 elem/lane/cycle @ 1.2 GHz regardless of dtype).
  * ctx: per-head [128,65] x [128,512] accumulating matmuls (vones carries a
    ones column; row 64 of ctx PSUM = softmax denominator).
- One FLAT software pipeline over all 256 (qb, k) units -- scores run 2 units
  ahead, crossing qb boundaries with no pipeline drain (PE idle gaps >3.4us
  trip the HAM MID window and throttle the PE clock 2.4->1.2 GHz).
- Out-projection of qb-1 and Q-projection of qb+1 are spread inside qb's
  unit loop at different k so PSUM slot reuse never stalls the PE on DVE.
- Per-unit PE work (~1.05us) ~= ACT exp (~1.11us): ACT-bound steady state
  with near-zero PE idle.
- PSUM: s2 [128,1024] x2 (4 banks) + ctx0 [128,512] x2 + ctx1 [128,512] x2
  (4 banks) = 8 banks.

TRN2 quirk: walrus encodes exactly ONE sync wait per TPB compute
instruction; _legalize_matmul_waits moves extra waits onto injected
single-wait same-engine no-ops.
"""

import sys
from contextlib import ExitStack

for _p in ("/opt/trn_rl_repo",):
    if _p not in sys.path:
        sys.path.insert(0, _p)

import numpy as np

import concourse.bass as bass
import concourse.tile as tile
from concourse import mybir
from concourse.bass_utils import run_bass_kernel_spmd

F32 = mybir.dt.float32
BF16 = mybir.dt.bfloat16
D = 512
DH = 64
P = 128
B = 2
S_FULL = 4096
N_CORES = 8
NC_T = D // P

LAST_RESULTS = None


def _emit(nc: bass.Bass, tc: "tile.TileContext", ctx: ExitStack, S: int):
    NK = S // P                  # 128-key tiles
    QB = 512 if S >= 512 else S
    NQB = S // QB                # q-blocks of 512
    U = NQB * NK                 # total pipeline units
    BS = min(1024, S)            # proj block size
    NB = S // BS
    KT_B = BS // P               # k-tiles per proj block
    inv_scale = 1.0 / np.sqrt(DH)

    def mm(out, lhsT, rhs, start=True, stop=True):
        return nc.tensor.matmul(out, lhsT, rhs, start=start, stop=stop)

    xt = nc.declare_dram_parameter("xt", [D, S], BF16, isOutput=False)
    wq = nc.declare_dram_parameter("wq", [D, P], BF16, isOutput=False)
    wk = nc.declare_dram_parameter("wk", [D, P], BF16, isOutput=False)
    wv = nc.declare_dram_parameter("wv", [D, P], BF16, isOutput=False)
    wo = nc.declare_dram_parameter("wo", [P, D], BF16, isOutput=False)
    yt = nc.declare_dram_parameter("yt", [D, S], F32, isOutput=True)

    const = ctx.enter_context(tc.tile_pool(name="const", bufs=1))

    # ---- input DMAs (block-major so proj block b can start early) ----
    w_sb = {n: [const.tile([P, P], BF16, tag=f"{n}{c}", name=f"{n}{c}")
                for c in range(NC_T)] for n in ("wq", "wk", "wv")}
    wo_sb = const.tile([P, D], BF16, tag="wo")
    xt_sb = [const.tile([P, S], BF16, tag=f"xt{c}", name=f"xt{c}")
             for c in range(NC_T)]
    wmap = {"wq": wq, "wk": wk, "wv": wv}

    def dma_w(name):
        for c in range(NC_T):
            nc.sync.dma_start(out=w_sb[name][c][:],
                              in_=wmap[name][c * P:(c + 1) * P, :])

    def dma_xt_block(b):
        bsl = slice(b * BS, (b + 1) * BS)
        for c in range(NC_T):
            nc.sync.dma_start(out=xt_sb[c][:, bsl], in_=xt[c * P:(c + 1) * P, bsl])

    dma_w("wv")
    dma_xt_block(0)
    dma_w("wk")
    dma_w("wq")
    nc.sync.dma_start(out=wo_sb[:], in_=wo[:, :])
    for b in range(1, NB):
        dma_xt_block(b)

    # ---- persistent SBUF ----
    qt_sb = const.tile([P, S], BF16, tag="qt")     # [2x64 dh, S]
    kt_sb = const.tile([P, S], BF16, tag="kt")
    vones = [const.tile([P, NK * (DH + 1)], BF16, tag=f"vones{h}", name=f"vones{h}")
             for h in range(2)]
    konst = const.tile([P, NK, 1], F32, tag="konst")
    nc.vector.memset(konst[:], 1.0)
    for h in range(2):
        vv = vones[h].rearrange("p (k c) -> p k c", c=DH + 1)
        nc.vector.tensor_copy(vv[:, :, DH:DH + 1], konst[:])
    ctx_sb = const.tile([P, S], BF16, tag="ctx")   # normalized context^T

    # PSUM pools: s2 2x[128,1024] (4 banks) + ctx0/ctx1 2x[128,512] each (4)
    ps = ctx.enter_context(tc.tile_pool(name="ps", bufs=2, space="PSUM"))
    es = ctx.enter_context(tc.tile_pool(name="es", bufs=3))
    bcp = ctx.enter_context(tc.tile_pool(name="bcp", bufs=2))
    rtp = ctx.enter_context(tc.tile_pool(name="rtp", bufs=2))
    rdp = ctx.enter_context(tc.tile_pool(name="rdp", bufs=2, space="DRAM"))
    osb = ctx.enter_context(tc.tile_pool(name="osb", bufs=2))

    # ---- phase A: V + K projections per block, Q for qb0(+qb1) ----
    def emit_vproj_block(b):
        pv = ps.tile([P, 2 * QB], F32, tag="s2", name=f"pv{b}")
        for j in range(KT_B):
            k = b * KT_B + j
            ksl = slice(k * P, (k + 1) * P)
            for c in range(NC_T):
                mm(pv[:, j * P:(j + 1) * P], xt_sb[c][:, ksl], w_sb["wv"][c][:],
                   start=(c == 0), stop=(c == NC_T - 1))
        for h in range(2):
            src = pv[:, 0:KT_B * P].rearrange("p (j c) -> p j c", c=P)[:, :, h * DH:(h + 1) * DH]
            dst = vones[h].rearrange("p (k c) -> p k c", c=DH + 1)[
                :, b * KT_B:(b + 1) * KT_B, 0:DH]
            nc.vector.tensor_copy(dst, src)

    def emit_kproj_block(b):
        bsl = slice(b * BS, (b + 1) * BS)
        pt = ps.tile([P, 2 * QB], F32, tag="s2", name=f"pk{b}")
        nh = max(1, BS // 512)
        hw_ = BS // nh
        for j in range(nh):
            jsl = slice(b * BS + j * hw_, b * BS + (j + 1) * hw_)
            for c in range(NC_T):
                mm(pt[:, j * hw_:(j + 1) * hw_], w_sb["wk"][c][:],
                   xt_sb[c][:, jsl], start=(c == 0), stop=(c == NC_T - 1))
        nc.vector.tensor_copy(kt_sb[:, bsl], pt[:, 0:BS])

    def emit_qproj_qb(qb):
        """Q projection for one 512-wide q-block into half an s2 tile."""
        qsl = slice(qb * QB, (qb + 1) * QB)
        pt = ps.tile([P, 2 * QB], F32, tag="s2", name=f"pq{qb}")
        for c in range(NC_T):
            mm(pt[:, 0:QB], w_sb["wq"][c][:], xt_sb[c][:, qsl],
               start=(c == 0), stop=(c == NC_T - 1))
        nc.vector.tensor_copy(qt_sb[:, qsl], pt[:, 0:QB])

    def emit_outproj_pair(qb, epair):
        """Out-projection for q-block qb, e-blocks (2*epair, 2*epair+1)."""
        qsl = slice(qb * QB, (qb + 1) * QB)
        o_ps = ps.tile([P, 2 * QB], F32, tag="s2", name=f"o{qb}_{epair}")
        for i in range(2):
            e = 2 * epair + i
            mm(o_ps[:, i * QB:(i + 1) * QB],
               wo_sb[:, e * P:(e + 1) * P], ctx_sb[:, qsl])
        o_sb = osb.tile([P, 2 * QB], F32, tag="osb", name="o_sb")
        nc.vector.tensor_copy(o_sb[:], o_ps[:, 0:2 * QB])
        for i in range(2):
            e = 2 * epair + i
            nc.sync.dma_start(out=yt[e * P:(e + 1) * P, qsl],
                              in_=o_sb[:, i * QB:(i + 1) * QB])

    for b in range(NB):
        emit_vproj_block(b)
        emit_kproj_block(b)
    emit_qproj_qb(0)
    if NQB > 1:
        emit_qproj_qb(1)

    # ---- attention: flat pipeline over units u = qb*NK + k ----
    def emit_scores_pair(u, s2):
        qb, k = divmod(u, NK)
        qsl = slice(qb * QB, (qb + 1) * QB)
        ksl = slice(k * P, (k + 1) * P)
        for h in range(2):
            hsl = slice(h * DH, (h + 1) * DH)
            mm(s2[:, h * QB:(h + 1) * QB], kt_sb[hsl, ksl], qt_sb[hsl, qsl])

    s_tiles = {}
    ctx_tiles = [None, None]     # live ctx PSUM tile per head
    for u in range(min(2, U)):
        s_tiles[u] = ps.tile([P, 2 * QB], F32, tag="s2", name=f"s{u}")
        emit_scores_pair(u, s_tiles[u])

    for u in range(U):
        qb, k = divmod(u, NK)
        if k == 0:
            for h in range(2):
                ctx_tiles[h] = ps.tile([P, QB], F32, tag=f"ctx{h}",
                                       name=f"ctx{qb}_{h}")
        # ACT: one exp for both heads
        e2 = es.tile([P, 2 * QB], BF16, tag="e", name=f"e{u}")
        nc.scalar.activation(e2[:], s_tiles.pop(u)[:],
                             mybir.ActivationFunctionType.Exp,
                             scale=inv_scale)
        # PE: ctx accumulation per head
        for h in range(2):
            vo = vones[h][:, k * (DH + 1):(k + 1) * (DH + 1)]
            mm(ctx_tiles[h][0:DH + 1, :], vo, e2[:, h * QB:(h + 1) * QB],
               start=(k == 0), stop=(k == NK - 1))
        # insertions (spread; never adjacent to each other)
        if k == 6 and qb + 2 < NQB:
            emit_qproj_qb(qb + 2)
        elif k == 14 and qb > 0:
            emit_outproj_pair(qb - 1, 0)
        elif k == 22 and qb > 0:
            emit_outproj_pair(qb - 1, 1)
        # PE: scores for unit u+2
        if u + 2 < U:
            s_tiles[u + 2] = ps.tile([P, 2 * QB], F32, tag="s2",
                                     name=f"s{u + 2}")
            emit_scores_pair(u + 2, s_tiles[u + 2])
        # drains at the end of each q-block
        if k == NK - 1:
            qsl = slice(qb * QB, (qb + 1) * QB)
            FW = QB // DH            # 8 free elems per lane
            # chain per head: row copy -> DRAM -> [64, FW] lanes -> 64-lane
            # recip -> DRAM -> partition-broadcast -> mul.  For the LAST
            # q-block both heads' chains are interleaved stage-by-stage so
            # their DMA latencies overlap (tail shortening); mid-kernel the
            # latency is hidden by ctx double-buffering anyway.
            heads = list(range(2))
            stages = [[] for _ in range(6)]
            for h in heads:
                cps = ctx_tiles[h]
                hsl = slice(h * DH, (h + 1) * DH)
                rt0 = rtp.tile([1, QB], F32, tag=f"rt0{h}", name=f"rt0{h}")
                rtT = rtp.tile([DH, FW], F32, tag=f"rt{h}", name=f"rt{h}")
                rtd = rdp.tile([1, QB], F32, tag=f"rtd{h}", name=f"rtd{h}")
                rtd2 = rdp.tile([1, QB], F32, tag=f"rtd2{h}", name=f"rtd2{h}")
                bc = bcp.tile([DH, QB], F32, tag=f"bc{h}", name=f"bc{h}")
                rtd_lanes = bass.AP(tensor=rtd.tensor, offset=rtd.offset,
                                    ap=[[FW, DH], [1, FW]])
                rtd2_lanes = bass.AP(tensor=rtd2.tensor, offset=rtd2.offset,
                                     ap=[[FW, DH], [1, FW]])
                rtd2_bcast = bass.AP(tensor=rtd2.tensor, offset=rtd2.offset,
                                     ap=[[0, DH], [1, QB]])
                stages[0].append(lambda cps=cps, rt0=rt0:
                    nc.vector.tensor_copy(rt0[0:1, :], cps[DH:DH + 1, :]))
                stages[1].append(lambda rt0=rt0, rtd=rtd:
                    nc.sync.dma_start(out=rtd[:], in_=rt0[0:1, :]))
                stages[2].append(lambda rtT=rtT, rtd_lanes=rtd_lanes: (
                    nc.sync.dma_start(out=rtT[:], in_=rtd_lanes),
                    nc.vector.reciprocal(rtT[:], rtT[:])))
                stages[3].append(lambda rtT=rtT, rtd2_lanes=rtd2_lanes:
                    nc.sync.dma_start(out=rtd2_lanes, in_=rtT[:]))
                stages[4].append(lambda bc=bc, rtd2_bcast=rtd2_bcast:
                    nc.sync.dma_start(out=bc[:], in_=rtd2_bcast))
                stages[5].append(lambda cps=cps, bc=bc, hsl=hsl:
                    nc.vector.tensor_mul(ctx_sb[hsl, qsl], cps[0:DH, :], bc[:]))
            if qb == NQB - 1:
                for st in stages:
                    for f in st:
                        f()
            else:
                for h in heads:
                    for st in stages:
                        st[h]()

    emit_outproj_pair(NQB - 1, 0)
    emit_outproj_pair(NQB - 1, 1)


def _legalize_matmul_waits(nc: bass.Bass) -> int:
    n_fixed = 0
    for f in nc.m.functions:
        for bb in f.blocks:
            out = []
            changed = False
            for ins in bb.instructions:
                si = ins.sync_info
                if (getattr(ins, "engine", None) is not None
                        and si is not None and len(si.on_wait) > 1):
                    for idx, w in enumerate(si.on_wait[:-1]):
                        nop = mybir.InstNoOp(name=f"{ins.name}-lgw{idx}",
                                             ins=[], outs=[])
                        nop.engine = ins.engine
                        nop.sync_info = mybir.SyncInfo(on_wait=[w], on_update=[])
                        out.append(nop)
                    ins.sync_info = mybir.SyncInfo(on_wait=[si.on_wait[-1]],
                                                   on_update=si.on_update)
                    n_fixed += 1
                    changed = True
                out.append(ins)
            if changed:
                bb.instructions = out
    return n_fixed


def build(S: int = S_FULL, legalize: bool = False) -> bass.Bass:
    nc = bass.Bass()
    with ExitStack() as ctx:
        ctx.enter_context(nc.allow_low_precision(
            reason="bf16 matmul operands; fp32 PSUM accumulation"))
        tc = ctx.enter_context(tile.TileContext(nc))
        _emit(nc, tc, ctx, S)
    if legalize:
        _legalize_matmul_waits(nc)
    return nc


_NC_CACHE = {}


def _get_nc(S: int) -> bass.Bass:
    if S not in _NC_CACHE:
        _NC_CACHE[S] = build(S, legalize=True)
    return _NC_CACHE[S]


def _to_bf16(a):
    import ml_dtypes
    return np.ascontiguousarray(a, dtype=np.float32).astype(ml_dtypes.bfloat16)


def make_in_maps(X, Wq, Wk, Wv, Wo):
    xts = [_to_bf16(X[b].T) for b in range(B)]
    in_maps = []
    for i in range(N_CORES):
        b, hp = divmod(i, 4)
        csl = slice(hp * P, (hp + 1) * P)
        in_maps.append({
            "xt": xts[b],
            "wq": _to_bf16(Wq[:, csl]),
            "wk": _to_bf16(Wk[:, csl]),
            "wv": _to_bf16(Wv[:, csl]),
            "wo": _to_bf16(Wo[csl, :]),
        })
    return in_maps


def kernel(X, Wq, Wk, Wv, Wo, _trace=False):
    global LAST_RESULTS
    X = np.asarray(X, dtype=np.float32)
    S = X.shape[1]
    nc = _get_nc(S)
    in_maps = make_in_maps(X, np.asarray(Wq, np.float32), np.asarray(Wk, np.float32),
                           np.asarray(Wv, np.float32), np.asarray(Wo, np.float32))
    res = run_bass_kernel_spmd(nc, in_maps, list(range(N_CORES)), trace=_trace)
    LAST_RESULTS = res
    Y = np.zeros((B, S, D), dtype=np.float32)
    for i in range(N_CORES):
        Y[i // 4] += res.results[i]["yt"].T
    return Y
